# revision 57
# baseline (speedup 1.0000x reference)
"""Trainium2 Bass kernel for nn_Attention: 16-head attention, B=2, S=2048, H=1024.

Strategy (Megatron tensor-parallel over heads, 8 cores x 2 heads), v2:
  - All device data in bf16 (fp32 PSUM accumulation), halving HBM traffic.
  - Transposed-context formulation: probabilities are the *stationary* matmul
    operand and V the moving one, so each context matmul streams only 65
    columns (64 dims + a ones column for the softmax denominator) instead of
    512 queries. Stationary loads are free on the PE, halving context cost.
  - V is produced directly in [token, dim] layout by per-token-chunk matmuls
    (x chunk stationary), eliminating the separate V transposes.
  - Normalization is a per-partition scalar multiply (denominator lands in
    the same partition as its query), then a single PE transpose per 128
    tokens feeds the dense projection.
  - The kc loop is software-pipelined (scores run one chunk ahead of the
    exp->context consumers) so the Activation engine never idles; psum->sbuf
    staging runs on DVE/Pool, keeping Act exclusively on exp.
  - Host sums the 8 partial dense outputs (Megatron all-reduce-after-dense),
    adds dense_b and the folded V-bias term.
"""
import os
import numpy as np
import ml_dtypes

B, S, H, NH = 2, 2048, 1024, 16
HD = H // NH            # 64
BS = B * S              # 4096
NCORES = 8
NK = H // 128           # 8 contraction chunks
NQB = S // 512          # 4 query windows per batch
NKC = S // 128          # 16 key chunks per batch

_CACHE = {}
_PHASE_LOG = []   # (label, next-instruction-id) markers, for profiling


def _bf16(x):
    return np.ascontiguousarray(x, dtype=np.float32).astype(ml_dtypes.bfloat16)


def _build_program():
    import concourse.mybir as mybir
    import concourse.tile as tile
    from concourse import bacc
    from contextlib import ExitStack

    F32 = mybir.dt.float32
    BF16 = mybir.dt.bfloat16
    Act = mybir.ActivationFunctionType

    nc = bacc.Bacc("TRN2", target_bir_lowering=False, debug=False,
                   num_devices=NCORES)
    xt = nc.dram_tensor("xt", [H, BS], BF16, kind="ExternalInput").ap()
    w1qk = nc.dram_tensor("w1qk", [H, 256], BF16, kind="ExternalInput").ap()
    # packed consts: crest1 = w1v (8x128 chunks); crest2 = eye | w2t
    crest1 = nc.dram_tensor("crest1", [128, 1024], BF16,
                            kind="ExternalInput").ap()
    crest2 = nc.dram_tensor("crest2", [128, 1152], BF16,
                            kind="ExternalInput").ap()
    qkb = nc.dram_tensor("qkb", [128, 2], F32, kind="ExternalInput").ap()
    out = nc.dram_tensor("out", [BS, H], BF16, kind="ExternalOutput").ap()
    debug = bool(int(os.environ.get("KERNEL_DEBUG", "0")))
    if debug:
        dbg_qt = nc.dram_tensor("dbg_qt", [128, BS], BF16,
                                kind="ExternalOutput").ap()
        dbg_kt = nc.dram_tensor("dbg_kt", [128, BS], BF16,
                                kind="ExternalOutput").ap()
        dbg_vsb = nc.dram_tensor("dbg_vsb", [128, 2 * 2 * NKC * 65], BF16,
                                 kind="ExternalOutput").ap()
        dbg_et = nc.dram_tensor("dbg_et", [128, 1024], BF16,
                                kind="ExternalOutput").ap()
        dbg_cxs = nc.dram_tensor("dbg_cxs", [128, 1024], F32,
                                 kind="ExternalOutput").ap()
        dbg_cts = nc.dram_tensor("dbg_cts", [128, 512], BF16,
                                 kind="ExternalOutput").ap()

    def _mark(label):
        _PHASE_LOG.append(
            (label, int(nc.get_next_instruction_name().split("-")[1])))

    with tile.TileContext(nc) as tc, nc.allow_low_precision(reason="bf16"):
        with ExitStack() as ctx:
            consts = ctx.enter_context(tc.tile_pool(name="consts", bufs=1))
            qkp = ctx.enter_context(tc.tile_pool(name="qkp", bufs=1))
            xtp = ctx.enter_context(tc.tile_pool(name="xtp", bufs=6))
            vsbp = ctx.enter_context(tc.tile_pool(name="vsbp", bufs=34))
            expp = ctx.enter_context(tc.tile_pool(name="expp", bufs=10))
            rsbp = ctx.enter_context(tc.tile_pool(name="rsbp", bufs=2))
            cxp = ctx.enter_context(tc.tile_pool(name="cxp", bufs=3))
            cnp = ctx.enter_context(tc.tile_pool(name="cnp", bufs=3))
            ctsp = ctx.enter_context(tc.tile_pool(name="ctsp", bufs=6))
            obp = ctx.enter_context(tc.tile_pool(name="obp", bufs=4))
            ps_sc = ctx.enter_context(tc.tile_pool(name="ps_sc", bufs=2,
                                                   space="PSUM"))
            ps_ctx = ctx.enter_context(tc.tile_pool(name="ps_ctx", bufs=2,
                                                    space="PSUM"))
            ps_ms = ctx.enter_context(tc.tile_pool(name="ps_ms", bufs=2,
                                                   space="PSUM"))

            # ---- constants ----
            w1qk_sb = consts.tile([128, NK, 256], BF16, name="w1qk")
            w1qk_r = w1qk.rearrange("(k p) m -> p k m", p=128)
            nc.sync.dma_start(w1qk_sb[:, 0:NK // 2, :], w1qk_r[:, 0:NK // 2, :])
            nc.sync.dma_start(w1qk_sb[:, NK // 2:NK, :], w1qk_r[:, NK // 2:NK, :])
            qkb_sb = consts.tile([128, 2], F32, name="qkb")
            nc.sync.dma_start(qkb_sb[:], qkb)
            warm = consts.tile([1, 1], F32, name="warm")
            nc.scalar.activation(warm[0:1, 0:1], qkb_sb[0:1, 0:1], Act.Exp)
            crest1_sb = consts.tile([128, 1024], BF16, name="crest1")
            crest2_sb = consts.tile([128, 1152], BF16, name="crest2")
            w1v_sb = crest1_sb[:, 0:1024].rearrange("p (k m) -> p k m", k=NK)
            eye_sb = crest2_sb[:, 0:128]
            w2_sb = crest2_sb[:, 128:1152]

            qt = qkp.tile([128, BS], BF16, name="qt")
            kt = qkp.tile([128, BS], BF16, name="kt")
            ones_sb = consts.tile([128, 1], BF16, name="ones")
            nc.vector.memset(ones_sb[:], 1.0)
            vsb = {}

            # ---- building blocks ----
            # Every PSUM tile's lifetime (alloc -> matmuls -> drain copy) is
            # emitted contiguously so the misc psum ring can never deadlock
            # on out-of-order buffer reuse.
            def emit_xt_dma(n, halves=False):
                _mark(f"xtdma{n}")
                xt_t = xtp.tile([128, NK, 512], BF16, name="xt")
                src = xt[:, n * 512:(n + 1) * 512].rearrange(
                    "(c p) f -> p c f", p=128)
                if halves:   # split by token half, matching the qk units
                    nc.sync.dma_start(xt_t[:, :, 0:256], src[:, :, 0:256])
                    nc.sync.dma_start(xt_t[:, :, 256:512], src[:, :, 256:512])
                else:
                    nc.sync.dma_start(xt_t[:], src)
                return xt_t

            def _ms_tile(shape, rr=False):
                return ps_ms.tile(shape, F32, name="psqk", tag="misc")

            def emit_qk_unit(n, m, xt_t, rr=False):
                """One projection unit: m=0 -> q rows, m=1 -> k rows.
                Two 256-token halves keep misc psum tiles at 1KB."""
                _mark(f"qk{n}.{'qk'[m]}")
                dst = qt if m == 0 else kt
                for h in (0, 1):
                    ps = _ms_tile([128, 256], rr)
                    for k in range(NK):
                        nc.tensor.matmul(ps[:],
                                         w1qk_sb[:, k, m * 128:(m + 1) * 128],
                                         xt_t[:, k, h * 256:(h + 1) * 256],
                                         start=(k == 0), stop=(k == NK - 1))
                    sl = slice(n * 512 + h * 256, n * 512 + (h + 1) * 256)
                    nc.vector.tensor_scalar_add(dst[:, sl], ps[:],
                                                qkb_sb[:, m:m + 1])

            def alloc_vsb(b):
                pass

            def emit_vT(xt_t, c4, t, rr=False):
                """V (+ones col) for global 128-token chunk t, both heads."""
                _mark(f"vT{t}")
                b, kc = divmod(t, NKC)
                ps = _ms_tile([128, 128], rr)
                for k in range(NK):
                    nc.tensor.matmul(ps[:],
                                     xt_t[:, k, c4 * 128:(c4 + 1) * 128],
                                     w1v_sb[:, k, :],
                                     start=(k == 0), stop=(k == NK - 1))
                vt_sb = vsbp.tile([128, 128], BF16, name="vsb")
                nc.vector.tensor_copy(vt_sb[:], ps[:])
                vsb[(b, kc)] = vt_sb

            def emit_scores_exp(b, qb, kc):
                _mark(f"se.{b}{qb}.{kc}")
                sp = ps_sc.tile([128, 1024], F32, name="sc", tag="sc")
                for j in (0, 1):
                    nc.tensor.matmul(
                        sp[:, j * 512:(j + 1) * 512],
                        kt[64 * j:64 * j + 64,
                           b * S + kc * 128:b * S + (kc + 1) * 128],
                        qt[64 * j:64 * j + 64,
                           b * S + qb * 512:b * S + (qb + 1) * 512],
                        start=True, stop=True)
                et = expp.tile([128, 1024], BF16, name="exp")
                nc.scalar.activation(et[:], sp[:], Act.Exp, scale=0.125)
                return et

            def emit_ctx(b, kc, et, ctxps):
                # start zeroes the whole psum bank, so only the first matmul
                # into each tile starts and only the last stops — the four
                # 65-col query groups all live in that one bank-group.
                _mark(f"cx.{b}.{kc}")
                for j in (0, 1):
                    for qc in range(4):
                        st = et[:, j * 512 + qc * 128:j * 512 + (qc + 1) * 128]
                        nc.tensor.matmul(
                            ctxps[j][:, qc * 128:qc * 128 + 64],
                            st,
                            vsb[(b, kc)][:, j * 64:(j + 1) * 64],
                            start=(kc == 0 and qc == 0),
                            stop=False, skip_group_check=True)
                        nc.tensor.matmul(
                            ctxps[j][:, qc * 128 + 64:qc * 128 + 65],
                            st, ones_sb[:],
                            start=False,
                            stop=(kc == NKC - 1 and qc == 3),
                            skip_group_check=True)

            def emit_norm(ctxps, dbg=False):
                """Drain ctx psum to sbuf fast (frees the accumulation ring
                for the next window), then normalize by the denominator
                column and transpose to [dims, tokens] off the critical
                path; returns the cts tile for the dense stage."""
                _mark("norm")
                cxs = {}
                for j in (0, 1):
                    cxs[j] = cxp.tile([128, 512], F32, name="cxs")
                    nc.vector.tensor_copy(cxs[j][:], ctxps[j][:])
                if dbg:
                    for j in (0, 1):
                        nc.sync.dma_start(dbg_cxs[:, j * 512:(j + 1) * 512],
                                          cxs[j][:])
                rt = rsbp.tile([128, 8, 1], F32, name="recip")
                for j in (0, 1):
                    dview = cxs[j][:].rearrange(
                        "p (g w) -> p g w", w=128)[:, :, 64:65]
                    nc.vector.reciprocal(rt[:, j * 4:(j + 1) * 4, :], dview)
                ctsw = ctsp.tile([128, 512], BF16, name="cts")
                for qc in range(4):
                    cn = cnp.tile([128, 128], BF16, name="cn")
                    for j in (0, 1):
                        nc.vector.tensor_scalar_mul(
                            cn[:, j * 64:(j + 1) * 64],
                            cxs[j][:, qc * 128:qc * 128 + 64],
                            rt[:, j * 4 + qc:j * 4 + qc + 1, 0])
                    pt = ps_ms.tile([128, 128], BF16, name="ctT", tag="misc")
                    nc.tensor.transpose(pt[:], cn[:], eye_sb)
                    nc.vector.tensor_copy(ctsw[:, qc * 128:(qc + 1) * 128],
                                          pt[:])
                if dbg:
                    nc.sync.dma_start(dbg_cts[:], ctsw[:])
                return ctsw

            def emit_dense(b, qb, qc, ctsw, pool=None, tag="misc",
                           split_dma=False, engs=None):
                _mark(f"dn.{b}{qb}.{qc}")
                pool = pool or ps_ms
                ob = obp.tile([128, H], BF16, name="ob")
                row0 = b * S + (qb * 4 + qc) * 128
                for nb in (0, 1):
                    dp = pool.tile([128, 512], F32, name="dp", tag=tag)
                    nc.tensor.matmul(dp[:], ctsw[:, qc * 128:(qc + 1) * 128],
                                     w2_sb[:, nb * 512:(nb + 1) * 512],
                                     start=True, stop=True)
                    sl = slice(nb * 512, (nb + 1) * 512)
                    if engs is not None and engs[(2 * qc + nb) % len(engs)] \
                            is nc.scalar:
                        nc.scalar.copy(ob[:, sl], dp[:])
                    else:
                        nc.vector.tensor_copy(ob[:, sl], dp[:])
                    if split_dma:
                        nc.sync.dma_start(
                            out[row0:row0 + 128, nb * 512:(nb + 1) * 512],
                            ob[:, nb * 512:(nb + 1) * 512])
                if not split_dma:
                    nc.sync.dma_start(out[row0:row0 + 128, :], ob[:])

            # ---- emission schedule ----
            # Per-window kc loop is pipelined one chunk ahead: slot kc emits
            # scores/exp(kc) then ctx(kc-1), so the PE always has the next
            # scores ready before Act finishes the current exp. qkv blocks
            # and vT chunks ride the PE slack inside the windows; the
            # previous window's norm runs at slot 0 and its dense chunks at
            # slots 1/5/9/13.
            # Prologue: only block-0 projection precedes window 0 —
            # everything else rides window slots so the scores/exp chain
            # (which paces the whole kernel) starts as early as possible.
            xts = {}
            xts[0] = emit_xt_dma(0, halves=True)
            nc.sync.dma_start(crest1_sb[:], crest1)
            xts[1] = emit_xt_dma(1)
            emit_qk_unit(0, 1, xts[0])   # k first: scores gate on kt
            emit_qk_unit(0, 0, xts[0])

            windows = [(b, qb) for b in range(B) for qb in range(NQB)]
            # window -> {slot: [hook, ...]}; hooks: ("dma", n) xt load,
            # ("crest2", 0), ("k"/"q", n) projection unit, ("vt", t) chunk.
            def _vt(t):
                return ("vt", t)

            hooks = {
                0: {0: [_vt(0)], 1: [("dma", 2), ("k", 1), _vt(1)],
                    2: [("q", 1), _vt(2)], 3: [_vt(3)], 4: [_vt(4)],
                    5: [("dma", 3), ("crest2", 0), _vt(5)],
                    6: [("k", 2), _vt(6)], 7: [_vt(7)],
                    8: [("q", 2), _vt(8)], 9: [_vt(9)],
                    10: [("k", 3), _vt(10)], 11: [_vt(11)],
                    12: [("q", 3), _vt(12)], 13: [_vt(13)], 14: [_vt(14)],
                    15: [_vt(15)]},
                1: {1: [("dma", 4)], 2: [_vt(16)], 4: [("k", 4)],
                    6: [_vt(17)], 8: [_vt(18)], 10: [_vt(19)]},
                2: {1: [("dma", 5)], 3: [_vt(20)], 4: [("k", 5)],
                    6: [_vt(21)], 8: [("q", 4)], 10: [_vt(22)],
                    12: [_vt(23)]},
                3: {1: [("dma", 6)], 3: [_vt(24)], 4: [("k", 6)],
                    6: [_vt(25)], 8: [_vt(26)], 10: [_vt(27)]},
                4: {1: [("dma", 7)], 3: [("k", 7)], 5: [_vt(28)],
                    6: [("q", 5)], 7: [_vt(29)], 9: [_vt(30)],
                    11: [_vt(31)]},
                5: {4: [("q", 6)]},
                6: {4: [("q", 7)]},
            }

            pend = None        # (b, qb, ctxps) awaiting norm
            cts_store = {}     # window -> (b, qb, ctsw) awaiting dense
            # window -> source windows whose dense chunks it emits
            dense_plan = {1: [0], 2: [], 3: [2], 4: [],
                          5: [1, 4], 6: [3, 5], 7: [6]}
            for w, (b, qb) in enumerate(windows):
                ctxps = {j: ps_ctx.tile([128, 512], F32,
                                        name=f"ctx{j}", tag="acc")
                         for j in (0, 1)}
                dq = []
                for i, srcw in enumerate(dense_plan.get(w, [])):
                    for qc in range(4):
                        dq.append((srcw, qc, 4 * i + qc))
                dslots = {2: 0, 6: 1, 10: 2, 14: 3,
                          4: 4, 8: 5, 12: 6, 15: 7}
                et_prev = None
                for kc in range(NKC):
                    et = emit_scores_exp(b, qb, kc)
                    if debug and w == 0 and kc == 0:
                        nc.sync.dma_start(dbg_et[:], et[:])
                    if kc == 0 and pend is not None:
                        pb, pq, pctx = pend
                        ctsw = emit_norm(pctx, dbg=(debug and w == 1))
                        cts_store[w - 1] = (pb, pq, ctsw)
                        pend = None
                    if kc > 0:
                        emit_ctx(b, kc - 1, et_prev, ctxps)
                    if kc in dslots:
                        for srcw, qc, idx in dq:
                            if idx == dslots[kc]:
                                sb_, sq_, sc_ = cts_store[srcw]
                                emit_dense(sb_, sq_, qc, sc_)
                    for hk in hooks.get(w, {}).get(kc, []):
                        what, n = hk
                        if what == "dma":
                            xts[n] = emit_xt_dma(n)
                        elif what == "crest2":
                            nc.sync.dma_start(crest2_sb[:], crest2)
                        elif what == "k":
                            emit_qk_unit(n, 1, xts[n])
                        elif what == "q":
                            emit_qk_unit(n, 0, xts[n])
                        else:
                            emit_vT(xts[n // 4], n % 4, n)
                    et_prev = et
                emit_ctx(b, NKC - 1, et_prev, ctxps)
                pend = (b, qb, ctxps)
            # emit remaining deferred dense (none expected) then the
            # final window's epilogue
            if debug:
                nc.sync.dma_start(dbg_qt[:], qt[:])
                nc.sync.dma_start(dbg_kt[:], kt[:])
                for kc_ in range(NKC):
                    nc.sync.dma_start(
                        dbg_vsb[:, kc_ * 128:(kc_ + 1) * 128],
                        vsb[(0, kc_)][:])
            # Final epilogue: split the norm across DVE and the now-idle
            # Act engine; dense cycles through both free psum pools.
            pb, pq, pctx = pend
            _mark("fin")
            cxs = {}
            for j in (0, 1):
                cxs[j] = cxp.tile([128, 512], F32, name="cxs")
                if j == 0:
                    nc.vector.tensor_copy(cxs[j][:], pctx[j][:])
                else:
                    nc.scalar.copy(cxs[j][:], pctx[j][:])
            rt = rsbp.tile([128, 8, 1], F32, name="recip")
            for j in (0, 1):
                dview = cxs[j][:].rearrange(
                    "p (g w) -> p g w", w=128)[:, :, 64:65]
                nc.vector.reciprocal(rt[:, j * 4:(j + 1) * 4, :], dview)
            ctsw = ctsp.tile([128, 512], BF16, name="cts")
            for qc in range(4):
                cn = cnp.tile([128, 128], BF16, name="cn")
                for j in (0, 1):
                    sc_ap = rt[:, j * 4 + qc:j * 4 + qc + 1, 0]
                    if j == 0:
                        nc.vector.tensor_scalar_mul(
                            cn[:, 0:64], cxs[0][:, qc * 128:qc * 128 + 64],
                            sc_ap)
                    else:
                        nc.scalar.mul(
                            cn[:, 64:128], cxs[1][:, qc * 128:qc * 128 + 64],
                            sc_ap)
                pt = ps_ms.tile([128, 128], BF16, name="ctT", tag="misc")
                nc.tensor.transpose(pt[:], cn[:], eye_sb)
                nc.vector.tensor_copy(ctsw[:, qc * 128:(qc + 1) * 128],
                                      pt[:])
            fin_pools = ((ps_sc, "sc"), (ps_sc, "sc"),
                         (ps_ctx, "acc"), (ps_ctx, "acc"))
            for qc in range(4):
                p_, t_ = fin_pools[qc % 4]
                emit_dense(pb, pq, qc, ctsw, pool=p_, tag=t_,
                           engs=(nc.vector, nc.scalar))
    nc.compile()
    return nc


def _prepare_inputs(hidden_states, qkv_w, qkv_b, dense_w):
    """Per-core host-side slicing/transposition/rounding."""
    x = np.ascontiguousarray(hidden_states, dtype=np.float32).reshape(BS, H)
    xt = _bf16(x.T)
    qkv_w = np.asarray(qkv_w, dtype=np.float32)
    qkv_b = np.asarray(qkv_b, dtype=np.float32)
    dense_w = np.asarray(dense_w, dtype=np.float32)
    in_maps = []
    for c in range(NCORES):
        h0, h1 = 2 * c, 2 * c + 1
        perm_qk = np.r_[h0 * 192:h0 * 192 + 64, h1 * 192:h1 * 192 + 64,
                        h0 * 192 + 64:h0 * 192 + 128,
                        h1 * 192 + 64:h1 * 192 + 128]
        perm_v = np.r_[h0 * 192 + 128:h0 * 192 + 192,
                       h1 * 192 + 128:h1 * 192 + 192]
        w1qk = _bf16(qkv_w[perm_qk, :].T)            # [1024, 256]
        w1v_t = qkv_w[perm_v, :].T                   # [1024, 128]
        # crest1[p, k*128 + m] = w1v_t[k*128 + p, m]
        crest1 = _bf16(np.ascontiguousarray(
            w1v_t.reshape(NK, 128, 128).transpose(1, 0, 2).reshape(128, 1024)))
        qkb = np.ascontiguousarray(
            np.stack([qkv_b[perm_qk[0:128]], qkv_b[perm_qk[128:256]]],
                     axis=1), dtype=np.float32)      # [128, 2]
        w2c = dense_w[:, c * 128:(c + 1) * 128].T    # [128, 1024]
        crest2 = _bf16(np.concatenate(
            [np.eye(128, dtype=np.float32), w2c], axis=1))  # [128, 1152]
        in_maps.append({
            "xt": xt, "w1qk": w1qk, "crest1": crest1, "crest2": crest2,
            "qkb": qkb,
        })
    return in_maps


def _reference_numpy(hidden_states, attention_mask, qkv_w, qkv_b, dense_w,
                     dense_b):
    """Exact fallback for non-all-ones masks (never hit with spec inputs)."""
    x = np.asarray(hidden_states, dtype=np.float64)
    mask = np.asarray(attention_mask, dtype=np.float64)
    mixed = x @ np.asarray(qkv_w, np.float64).T + np.asarray(qkv_b, np.float64)
    mixed = mixed.reshape(B, S, NH, 3 * HD).transpose(0, 2, 1, 3)
    q, k, v = np.split(mixed, 3, axis=-1)
    scores = np.einsum("bhqd,bhkd->bhqk", q, k) / np.sqrt(HD)
    scores = scores * mask - 10000.0 * (1.0 - mask)
    scores -= scores.max(axis=-1, keepdims=True)
    probs = np.exp(scores)
    probs /= probs.sum(axis=-1, keepdims=True)
    cx = np.einsum("bhqk,bhkd->bhqd", probs, v)
    cx = cx.transpose(0, 2, 1, 3).reshape(B, S, H)
    o = cx @ np.asarray(dense_w, np.float64).T + np.asarray(dense_b, np.float64)
    return o.astype(np.float32)


def _run(inputs, trace=False):
    from concourse.bass_utils import run_bass_kernel_spmd
    if "nc" not in _CACHE:
        _CACHE["nc"] = _build_program()
    nc = _CACHE["nc"]
    in_maps = _prepare_inputs(inputs["hidden_states"], inputs["qkv_w"],
                              inputs["qkv_b"], inputs["dense_w"])
    res = run_bass_kernel_spmd(nc, in_maps, core_ids=list(range(NCORES)),
                               trace=trace)
    partials = np.stack([np.asarray(r["out"], dtype=np.float64)
                         for r in res.results], axis=0)
    full = partials.sum(axis=0)
    qkv_b = np.asarray(inputs["qkv_b"], dtype=np.float64)
    dense_w = np.asarray(inputs["dense_w"], dtype=np.float64)
    # v-bias folding: ctx = sum_k p_k (v_k + b_v) = sum_k p_k v_k + b_v
    b_v = np.concatenate([qkv_b[h * 192 + 128:h * 192 + 192]
                          for h in range(NH)])
    full += np.asarray(inputs["dense_b"], dtype=np.float64) + dense_w @ b_v
    return full.astype(np.float32).reshape(B, S, H), res


def kernel(hidden_states, attention_mask, qkv_w, qkv_b, dense_w, dense_b):
    hidden_states = np.asarray(hidden_states)
    attention_mask = np.asarray(attention_mask)
    qkv_w = np.asarray(qkv_w)
    qkv_b = np.asarray(qkv_b)
    dense_w = np.asarray(dense_w)
    dense_b = np.asarray(dense_b)
    if not np.all(attention_mask == 1.0):
        return _reference_numpy(hidden_states, attention_mask, qkv_w, qkv_b,
                                dense_w, dense_b)
    out, _ = _run({
        "hidden_states": hidden_states, "qkv_w": qkv_w, "qkv_b": qkv_b,
        "dense_w": dense_w, "dense_b": dense_b,
    }, trace=bool(int(os.environ.get("KERNEL_TRACE", "0"))))
    return out
